# revision 1
# baseline (speedup 1.0000x reference)
"""AtIndexPooler (embedding lookup) on 8 TRN2 NeuronCores.

Data-parallel along batch: each core owns B/8 = 64 batch rows. Per core the
hidden_state shard is viewed as a flat row table [64*512, 1024] with the two
missing-embedding rows appended at the end ([32770, 1024] total). The host
turns indices into flat row offsets (invalid index -1 -> appended missing
row), and the device performs the gather with one indirect DMA of 128 rows
(one per SBUF partition) followed by a contiguous store of the pooled output.
"""

import sys

import numpy as np

if "/opt/trn_rl_repo" not in sys.path:
    sys.path.insert(0, "/opt/trn_rl_repo")

from concourse import bacc, bass, mybir, tile
from concourse.bass_utils import run_bass_kernel_spmd

BATCH, SEQ_LEN, HIDDEN = 512, 512, 1024
NUM_INDICES = 2
N_CORES = 8
B_SHARD = BATCH // N_CORES                # 64 batches per core
ROWS = B_SHARD * NUM_INDICES              # 128 gather rows = 128 partitions
DATA_ROWS = B_SHARD * SEQ_LEN + NUM_INDICES  # 32770 rows in the lookup table

_NC_CACHE = None
LAST_RESULT = None  # BassKernelResults of the most recent run (for profiling)


def _build_nc():
    nc = bacc.Bacc("TRN2", target_bir_lowering=False, debug=False, num_devices=N_CORES)
    data = nc.dram_tensor("data", [DATA_ROWS, HIDDEN], mybir.dt.float32, kind="ExternalInput")
    offs = nc.dram_tensor("offs", [ROWS, 1], mybir.dt.int32, kind="ExternalInput")
    out = nc.dram_tensor("out", [ROWS, HIDDEN], mybir.dt.float32, kind="ExternalOutput")

    with tile.TileContext(nc) as tc:
        with tc.tile_pool(name="sbuf", bufs=1) as pool:
            offs_sb = pool.tile([ROWS, 1], mybir.dt.int32)
            gathered = pool.tile([ROWS, HIDDEN], mybir.dt.float32)
            nc.sync.dma_start(out=offs_sb[:], in_=offs[:, :])
            nc.gpsimd.indirect_dma_start(
                out=gathered[:],
                out_offset=None,
                in_=data[:, :],
                in_offset=bass.IndirectOffsetOnAxis(ap=offs_sb[:, :1], axis=0),
            )
            nc.sync.dma_start(out=out[:, :], in_=gathered[:])

    nc.compile()
    return nc


def kernel(hidden_state, missing_embeddings, indices):
    global _NC_CACHE, LAST_RESULT
    hidden_state = np.ascontiguousarray(np.asarray(hidden_state, dtype=np.float32))
    missing_embeddings = np.ascontiguousarray(np.asarray(missing_embeddings, dtype=np.float32))
    indices = np.asarray(indices)

    if _NC_CACHE is None:
        _NC_CACHE = _build_nc()
    nc = _NC_CACHE

    base = (np.arange(B_SHARD, dtype=np.int64) * SEQ_LEN)[:, None]
    miss_rows = B_SHARD * SEQ_LEN + np.arange(NUM_INDICES, dtype=np.int64)[None, :]

    in_maps = []
    for c in range(N_CORES):
        hs = hidden_state[c * B_SHARD : (c + 1) * B_SHARD].reshape(B_SHARD * SEQ_LEN, HIDDEN)
        idx = indices[c * B_SHARD : (c + 1) * B_SHARD].astype(np.int64)  # [64, 2]
        flat = np.where(idx >= 0, base + np.clip(idx, 0, SEQ_LEN - 1), miss_rows)
        offs = flat.astype(np.int32).reshape(ROWS, 1)
        data = np.concatenate([hs, missing_embeddings], axis=0)
        in_maps.append({"data": data, "offs": offs})

    LAST_RESULT = run_bass_kernel_spmd(nc, in_maps, core_ids=list(range(N_CORES)))
    outs = [
        LAST_RESULT.results[c]["out"].reshape(B_SHARD, NUM_INDICES * HIDDEN)
        for c in range(N_CORES)
    ]
    return np.concatenate(outs, axis=0)


# revision 4
# speedup vs baseline: 1.1115x; 1.1115x over previous
"""AtIndexPooler (embedding lookup) on 8 TRN2 NeuronCores.

Data-parallel along batch: each core owns B/8 = 64 batch rows. Per core the
hidden_state shard is viewed as a flat row table [64*512, 1024] with the two
missing-embedding rows appended at the end ([32770, 1024] total). The host
turns indices into flat row offsets (invalid index -1 -> appended missing
row), and the device performs the gather with one indirect DMA of 128 rows
(one per SBUF partition) followed by a contiguous store of the pooled output.
"""

import sys

import numpy as np

if "/opt/trn_rl_repo" not in sys.path:
    sys.path.insert(0, "/opt/trn_rl_repo")

from concourse import bacc, bass, mybir, tile
from concourse.bass_utils import run_bass_kernel_spmd

BATCH, SEQ_LEN, HIDDEN = 512, 512, 1024
NUM_INDICES = 2
N_CORES = 8
B_SHARD = BATCH // N_CORES                # 64 batches per core
ROWS = B_SHARD * NUM_INDICES              # 128 gather rows = 128 partitions
DATA_ROWS = B_SHARD * SEQ_LEN + NUM_INDICES  # 32770 rows in the lookup table

_NC_CACHE = None
LAST_RESULT = None  # BassKernelResults of the most recent run (for profiling)


def _build_nc():
    nc = bacc.Bacc("TRN2", target_bir_lowering=False, debug=False, num_devices=N_CORES)
    data = nc.dram_tensor("data", [DATA_ROWS, HIDDEN], mybir.dt.float32, kind="ExternalInput")
    offs = nc.dram_tensor("offs", [ROWS, 1], mybir.dt.int32, kind="ExternalInput")
    out = nc.dram_tensor("out", [ROWS, HIDDEN], mybir.dt.float32, kind="ExternalOutput")

    sA = nc.alloc_semaphore("sA")  # offs load completion
    sB = nc.alloc_semaphore("sB")  # indirect gather completion
    sC = nc.alloc_semaphore("sC")  # output store completion
    offs_sb = nc.alloc_sbuf_tensor("offs_sb", [ROWS, 1], mybir.dt.int32)
    gath = nc.alloc_sbuf_tensor("gath", [ROWS, HIDDEN], mybir.dt.float32)

    nc.sync.dma_start(out=offs_sb[:, :], in_=offs[:, :]).then_inc(sA, 16)
    nc.gpsimd.wait_ge(sA, 16)
    nc.gpsimd.indirect_dma_start(
        out=gath[:, :],
        out_offset=None,
        in_=data[:, :],
        in_offset=bass.IndirectOffsetOnAxis(ap=offs_sb[:, :1], axis=0),
    ).then_inc(sB, 16)
    nc.sync.wait_ge(sB, 16)
    nc.sync.dma_start(out=out[:, :], in_=gath[:, :]).then_inc(sC, 16)
    nc.sync.wait_ge(sC, 16)
    for s in (sA, sB, sC):
        nc.sync.sem_clear(s)

    nc.compile()
    return nc


def kernel(hidden_state, missing_embeddings, indices):
    global _NC_CACHE, LAST_RESULT
    hidden_state = np.ascontiguousarray(np.asarray(hidden_state, dtype=np.float32))
    missing_embeddings = np.ascontiguousarray(np.asarray(missing_embeddings, dtype=np.float32))
    indices = np.asarray(indices)

    if _NC_CACHE is None:
        _NC_CACHE = _build_nc()
    nc = _NC_CACHE

    base = (np.arange(B_SHARD, dtype=np.int64) * SEQ_LEN)[:, None]
    miss_rows = B_SHARD * SEQ_LEN + np.arange(NUM_INDICES, dtype=np.int64)[None, :]

    in_maps = []
    for c in range(N_CORES):
        hs = hidden_state[c * B_SHARD : (c + 1) * B_SHARD].reshape(B_SHARD * SEQ_LEN, HIDDEN)
        idx = indices[c * B_SHARD : (c + 1) * B_SHARD].astype(np.int64)  # [64, 2]
        flat = np.where(idx >= 0, base + np.clip(idx, 0, SEQ_LEN - 1), miss_rows)
        offs = flat.astype(np.int32).reshape(ROWS, 1)
        data = np.concatenate([hs, missing_embeddings], axis=0)
        in_maps.append({"data": data, "offs": offs})

    LAST_RESULT = run_bass_kernel_spmd(nc, in_maps, core_ids=list(range(N_CORES)))
    outs = [
        LAST_RESULT.results[c]["out"].reshape(B_SHARD, NUM_INDICES * HIDDEN)
        for c in range(N_CORES)
    ]
    return np.concatenate(outs, axis=0)
